# revision 26
# baseline (speedup 1.0000x reference)
"""CosformerAttention (causal linear attention) Trainium2 Bass kernel.

Full inputs in, full output out. Shards batch*heads over 8 NeuronCores:
device d handles sample n = d//4 and heads hA = 2*(d%4), hB = hA+1.

Lean dataflow (v3):
  - q/k projected UNDOUBLED feature-major (128 = 2h x 64 feats, L) - the
    cos/sin reweighting identity (doubled q_i . doubled k_j ==
    cos(th_i-th_j) * q_i.k_j) is folded into the causal mask, so
    intra-chunk scores contract over 64 plain features per head.
  - doubled q (inter-chunk state matmul stationary) built by a PE
    dup-matmul + one DVE scale by the sin/cos table.
  - v projected feature-major, then PE-transposed per chunk to seq-major;
    k seq-major likewise (s/c scaling split across ACT and DVE).
  - prefix sum of chunk states on the otherwise-idle GpSimd.
  - inputs split/ordered so the first projection starts ~2.5us after the
    fixed ~6.8us engine-init prologue (DMA triggers hoisted to program
    head); bf16 output partials, host sums 4 per-sample partials in f32.

Self-contained: hardcodes L=1024, N=2, E=512, H=8 from the problem spec.
"""

import sys

if "/opt/trn_rl_repo" not in sys.path:
    sys.path.insert(0, "/opt/trn_rl_repo")

import numpy as np
import ml_dtypes

BF16NP = ml_dtypes.bfloat16

import concourse.bass as bass
import concourse.tile as tile
from concourse import mybir
import concourse.bass_utils as bass_utils
from concourse.vector_clock import ScopedClock

F32 = mybir.dt.float32
BF16 = mybir.dt.bfloat16
ALU = mybir.AluOpType
ACTF = mybir.ActivationFunctionType

L, N, E, H = 1024, 2, 512, 8
D = E // H          # 64 head dim
P = 128             # partitions / chunk size
NCHUNK = L // P     # 8
NCORES = 8
EPS = 1e-6
TCH = 512           # seq half handled per xT tile

# wbA (bf16) column offsets: stationaries needed by the first matmuls
WQK = 0            # (4, 256) e-major [qA qB kA kB]
DUP = 1024         # (2, 128) dup_h[p, f] = (p == h*64 + f%64)
WACOLS = 1280
# wbB (bf16) column offsets
WV = 0             # (4, 128) e-major [vA vB]
OUTW = 512         # (512,)
IDENT = 1024       # (128,)
SC = 1152          # (1024,) rows 0:64 = sin, 64:128 = cos
WBCOLS = 2176
# cpack (f32) column offsets
MASK = 0           # (128,) cosmask
SCOL = 128         # (8,)
CCOL = 136         # (8,)
QB = 144
KB = 145
VB = 146
CPCOLS = 147


# ---------------------------------------------------------------------------
# This walrus build allows at most ONE semaphore wait per instruction.
# (a) Tile's tail drain carries the whole global clock: split it across
#     preceding SP nops.  (b) Skip the tail barriers + semaphore clearing --
#     the Bass preamble already dma_resets + sem_clears the entire kernel
#     semaphore range at program start, so end-of-kernel cleanup is
#     redundant and costs ~10us of EVSEM butterfly.
# ---------------------------------------------------------------------------
def _patched_drain_and_barrier(self, tick_clock, wait_clock):
    nc = self.nc
    nops = [nc.sync.nop() for _ in range(48)]
    drain_inst = nc.sync.drain()
    wait_clock.add_sem_waits(
        drain_inst.ins, ScopedClock({None: tick_clock.global_clock})
    )
    waits = list(drain_inst.ins.sync_info.on_wait or [])
    if len(waits) > 1:
        drain_inst.ins.sync_info.on_wait = [waits[-1]]
        SI = type(drain_inst.ins.sync_info)
        for nop, w in zip(nops, waits[:-1]):
            si = nop.ins.sync_info
            if si is None:
                nop.ins.sync_info = SI(on_wait=[w], on_update=[])
            else:
                si.on_wait = [w]
    nc.all_engine_barrier()
    popped = nc._tile_sem_poison_stack.pop()
    assert popped is self._sem_poison


tile.TileContext._drain_and_barrier = _patched_drain_and_barrier


def _split_multi_waits(nc):
    """Move excess sem waits onto preceding same-engine NoOps (engines
    execute strictly in order, so this is equivalent)."""
    k = 0
    for f in nc.m.functions:
        for bb in f.blocks:
            insts = list(bb.instructions)
            out, changed = [], False
            for inst in insts:
                si = inst.sync_info
                waits = list(si.on_wait) if (si is not None and si.on_wait) else []
                if len(waits) > 1 and "Unassigned" not in str(inst.engine):
                    for w in waits[:-1]:
                        nop = mybir.InstNoOp(name=f"wsplit-{k}", ins=[], outs=[])
                        k += 1
                        nop.engine = inst.engine
                        nop.sync_info = type(si)(on_wait=[w], on_update=[])
                        out.append(nop)
                    si.on_wait = [waits[-1]]
                    changed = True
                out.append(inst)
            if changed:
                bb.instructions = out


def _hoist_input_dmas(nc, n_inputs):
    """Move the first n_inputs InstDMACopy (the input loads, which have no
    waits) from the tile block to the head of the main block, so the input
    DMA overlaps the engine-init prologue."""
    blocks = [bb for f in nc.m.functions for bb in f.blocks]
    main = next(bb for bb in blocks if bb.name == "main")
    tb = next(bb for bb in blocks if bb.name.startswith("tile_context"))
    moved, rest = [], []
    for inst in tb.instructions:
        if (len(moved) < n_inputs and type(inst).__name__ == "InstDMACopy"
                and not (inst.sync_info and inst.sync_info.on_wait)):
            moved.append(inst)
        else:
            rest.append(inst)
    assert len(moved) == n_inputs, f"found {len(moved)} input DMAs"
    tb.instructions = rest
    main.instructions = moved + list(main.instructions)


def bcast(ap, dims):
    """Append broadcast (step 0) free dims to an AP."""
    return bass.AP(tensor=ap.tensor, offset=ap.offset,
                   ap=list(ap.ap) + [[0, d] for d in dims])


def mid_bcast(ap, n):
    """Insert a step-0 dim of size n between partition dim and free dims."""
    return bass.AP(tensor=ap.tensor, offset=ap.offset,
                   ap=[ap.ap[0], [0, n]] + list(ap.ap[1:]))


def build_program(hoist=True):
    nc = bass.Bass("TRN2", target_bir_lowering=False)

    # ---- DRAM I/O (layouts match SBUF tiles exactly) -----------------------
    wa_d = nc.dram_tensor("wa", [P, WACOLS], BF16, kind="ExternalInput").ap()
    x0_d = nc.dram_tensor("x0", [P, 4, TCH], BF16, kind="ExternalInput").ap()
    wb_d = nc.dram_tensor("wb", [P, WBCOLS], BF16, kind="ExternalInput").ap()
    x1_d = nc.dram_tensor("x1", [P, 4, TCH], BF16, kind="ExternalInput").ap()
    cp_d = nc.dram_tensor("cp", [P, CPCOLS], F32, kind="ExternalInput").ap()
    out_d = nc.dram_tensor("out", [L, E], BF16, kind="ExternalOutput").ap()

    with tile.TileContext(nc) as tc:
        persist = tc.alloc_tile_pool(name="persist", bufs=1)
        work = tc.alloc_tile_pool(name="work", bufs=4)
        small = tc.alloc_tile_pool(name="small", bufs=4)
        ps_big = tc.alloc_tile_pool(name="ps_big", bufs=2, space="PSUM")
        ps_sq = tc.alloc_tile_pool(name="ps_sq", bufs=2, space="PSUM")
        ps_tp = tc.alloc_tile_pool(name="ps_tp", bufs=2, space="PSUM")
        ps_po = tc.alloc_tile_pool(name="ps_po", bufs=2, space="PSUM")

        # ---- input loads (hoisted to program head post-build), in the
        # order the compute consumes them ------------------------------------
        wa = persist.tile([P, WACOLS], BF16, tag="wa", name="wa")
        nc.scalar.dma_start(out=wa[:], in_=wa_d)
        xT = [persist.tile([P, 4, TCH], BF16, tag=f"x{t}", name=f"x{t}")
              for t in range(2)]
        nc.sync.dma_start(out=xT[0][:], in_=x0_d)
        nc.scalar.dma_start(out=xT[1][:], in_=x1_d)
        wb = persist.tile([P, WBCOLS], BF16, tag="wb", name="wb")
        nc.sync.dma_start(out=wb[:], in_=wb_d)
        cp = persist.tile([P, CPCOLS], F32, tag="cp", name="cp")
        nc.scalar.dma_start(out=cp[:], in_=cp_d)

        identv = wb[:, IDENT:IDENT + P]
        outw = wb[:, OUTW:OUTW + E]
        cosmask = cp[:, MASK:MASK + P]

        # persistent activations (split per seq-half for finer DMA overlap)
        q_p = [persist.tile([P, TCH], BF16, tag=f"qp{t}", name=f"qp{t}")
               for t in range(2)]
        k_p = [persist.tile([P, TCH], BF16, tag=f"kp{t}", name=f"kp{t}")
               for t in range(2)]
        v_fm = [persist.tile([P, TCH], BF16, tag=f"vfm{t}", name=f"vfm{t}")
                for t in range(2)]
        q_f = [persist.tile([P, L], BF16, tag=f"qf{h}", name=f"qf{h}")
               for h in range(2)]
        # k_t: [ch, head, sc, d] sequence-layout scaled k
        k_t = persist.tile([P, NCHUNK, 2, 2, D], BF16, tag="kt", name="kt")
        # v_t: [ch, head, d+1] with ones column
        v_t = persist.tile([P, NCHUNK, 2, D + 1], BF16, tag="vt", name="vt")
        attn = persist.tile([P, NCHUNK, P], BF16, tag="attn", name="attn")
        Sc_sb = persist.tile([P, NCHUNK, 2, D + 1], BF16, tag="scsb", name="scsb")
        Spfx = persist.tile([P, NCHUNK, 2, D + 1], BF16, tag="spfx", name="spfx")

        nc.vector.memset(v_t[:, :, :, D:D + 1], 1.0)

        # PE warm-up: dummy matmuls on uninitialized SBUF keep the PE busy
        # (HAM at full clock) while the input DMA streams in; results unused.
        warm = persist.tile([P, 640], BF16, tag="warm", name="warm")
        nc.gpsimd.memset(warm[:], 0.125)
        for w in range(10):
            psw = ps_big.tile([P, TCH], F32, tag="big", name="psw")
            nc.tensor.matmul(psw[:], warm[:, 0:128], warm[:, 128:640],
                             start=True, stop=True)

        # ---- stage B: feature-major q/k/v projections ----------------------
        def proj(t, wlo, bias_col, actf, dst):
            ps = ps_big.tile([P, TCH], F32, tag="big", name="ps")
            for e in range(4):
                nc.tensor.matmul(
                    ps[:], wa[:, WQK + e * 256 + wlo: WQK + e * 256 + wlo + P]
                    if wlo < 256 else
                    wb[:, WV + e * P: WV + (e + 1) * P],
                    xT[t][:, e, :], start=(e == 0), stop=(e == 3))
            nc.scalar.activation(dst[t][:], ps[:], actf,
                                 bias=cp[:, bias_col:bias_col + 1], scale=1.0)

        def dup_q(t):
            for h in range(2):
                psd = ps_big.tile([P, TCH], F32, tag="big", name="psd")
                nc.tensor.matmul(psd[:], wa[:, DUP + h * P:DUP + (h + 1) * P],
                                 q_p[t][:], start=True, stop=True)
                nc.vector.tensor_mul(
                    q_f[h][:, t * TCH:(t + 1) * TCH], psd[:],
                    wb[:, SC + t * TCH:SC + (t + 1) * TCH])

        for t in range(2):
            proj(t, 0, QB, ACTF.Relu, q_p)       # q
            proj(t, 128, KB, ACTF.Relu, k_p)     # k
            dup_q(t)
            proj(t, 999, VB, ACTF.Identity, v_fm)  # v (wlo>=256 -> wv)

        # ---- stage C + D1 fused per chunk ----------------------------------
        for ch in range(NCHUNK):
            t, cs = ch // 4, slice((ch % 4) * P, (ch % 4 + 1) * P)
            ptk = ps_tp.tile([P, P], BF16, tag="tp", name="ptk")
            nc.tensor.transpose(ptk[:], k_p[t][:, cs], identv)
            kc = ptk[:].rearrange("p (h d) -> p h d", h=2)
            nc.scalar.activation(k_t[:, ch, :, 0, :], kc, ACTF.Relu,
                                 scale=cp[:, SCOL + ch:SCOL + ch + 1])
            ccol = cp[:, CCOL + ch:CCOL + ch + 1]
            cbc = bass.AP(tensor=ccol.tensor, offset=ccol.offset,
                          ap=[ccol.ap[0], [0, 2 * D]])
            nc.vector.tensor_mul(k_t[:, ch, :, 1, :], kc, cbc)
            ptv = ps_tp.tile([P, P], BF16, tag="tp", name="ptv")
            nc.tensor.transpose(ptv[:], v_fm[t][:, cs], identv)
            nc.vector.tensor_copy(
                v_t[:, ch, :, 0:D],
                ptv[:].rearrange("p (h d) -> p h d", h=2))
            psc = ps_po.tile([P, 2, D + 1], F32, tag="po130", name="psc")
            for h in range(2):
                nc.tensor.matmul(psc[:, h, :], k_t[:, ch, h, :, :],
                                 v_t[:, ch, h, :], start=True, stop=True)
            if ch % 2 == 0:
                nc.scalar.activation(Sc_sb[:, ch, :, :], psc[:], ACTF.Copy)
            else:
                nc.vector.tensor_copy(Sc_sb[:, ch, :, :], psc[:])
            if ch == 1:
                nc.gpsimd.tensor_copy(Spfx[:, 1], Sc_sb[:, 0])
            elif ch > 1:
                nc.gpsimd.tensor_add(Spfx[:, ch], Spfx[:, ch - 1],
                                     Sc_sb[:, ch - 1])

        # ---- stage D2 + E: per-chunk attention, lag-one output proj --------
        osb_ref = [None]

        def stage_e_head(ch):
            pst = ps_tp.tile([P, P], BF16, tag="tp", name="pst")
            nc.tensor.transpose(pst[:], attn[:, ch, :], identv)
            aTw = work.tile([P, P], BF16, tag="aT", name="aTw")
            nc.vector.tensor_copy(aTw[:], pst[:])
            return aTw

        def stage_e_tail(ch, aTw):
            g, j = ch // 2, ch % 2
            pso = ps_big.tile([P, E], F32, tag="big", name="pso")
            nc.tensor.matmul(pso[:], aTw[:], outw, start=True, stop=True)
            if j == 0:
                osb_ref[0] = work.tile([P, 2, E], BF16, tag="osb", name="osb")
            nc.scalar.activation(osb_ref[0][:, j, :], pso[:], ACTF.Copy)
            if j == 1:
                nc.sync.dma_start(
                    out=out_d[g * 2 * P:(g + 1) * 2 * P, :].rearrange(
                        "(j p) e -> p j e", p=P),
                    in_=osb_ref[0][:])

        aT_prev = None
        for ch in range(NCHUNK):
            t, cs = ch // 4, slice((ch % 4) * P, (ch % 4 + 1) * P)
            ms = work.tile([P, 2, P], BF16, tag="ms", name="ms")
            for h in range(2):
                pss = ps_sq.tile([P, P], F32, tag="sq", name="pss")
                nc.tensor.matmul(pss[:], k_p[t][h * D:(h + 1) * D, cs],
                                 q_p[t][h * D:(h + 1) * D, cs],
                                 start=True, stop=True)
                nc.vector.tensor_mul(ms[:, h, :], pss[:], cosmask)
            if ch >= 1:
                aT_prev = stage_e_head(ch - 1)
            po = ps_po.tile([P, 2, D + 1], F32, tag="po130", name="po")
            for h in range(2):
                nc.tensor.matmul(po[:, h, :], ms[:, h, :], v_t[:, ch, h, :],
                                 start=True, stop=(ch == 0))
                if ch > 0:
                    nc.tensor.matmul(po[:, h, :], q_f[h][:, ch * P:(ch + 1) * P],
                                     Spfx[:, ch, h, :], start=False, stop=True)
            if ch >= 1:
                stage_e_tail(ch - 1, aT_prev)
            rec = small.tile([P, 2], F32, tag="rec", name="rec")
            nc.vector.reciprocal(rec[:], po[:, :, D])
            nc.vector.tensor_mul(
                attn[:, ch, :].rearrange("p (h d) -> p h d", h=2),
                po[:, :, 0:D],
                bcast(rec[:, :], [D]),
            )
        stage_e_tail(NCHUNK - 1, stage_e_head(NCHUNK - 1))

        for p in (ps_po, ps_tp, ps_sq, ps_big, small, work, persist):
            p.release()

    _split_multi_waits(nc)
    if hoist:
        _hoist_input_dmas(nc, 5)
    return nc


_PROG = {}


def _get_program():
    if "nc" not in _PROG:
        _PROG["nc"] = build_program()
    return _PROG["nc"]


_CONST = {}


def _const_tables():
    if not _CONST:
        idx = np.arange(1, L + 1, dtype=np.float64) * (np.pi / 2) / L
        s, c = np.sin(idx), np.cos(idx)
        _CONST["sc"] = np.concatenate(
            [np.broadcast_to(s, (D, L)), np.broadcast_to(c, (D, L))],
            axis=0).astype(BF16NP).astype(np.float32)
        jj, ii = np.meshgrid(np.arange(P), np.arange(P), indexing="ij")
        _CONST["cosmask"] = (
            np.cos((np.pi / 2) * (ii - jj) / L) * (jj <= ii)).astype(np.float32)
        _CONST["s_col"] = np.ascontiguousarray(
            s.reshape(NCHUNK, P).T).astype(np.float32)
        _CONST["c_col"] = np.ascontiguousarray(
            c.reshape(NCHUNK, P).T).astype(np.float32)
        _CONST["ident"] = np.eye(P, dtype=np.float32)
        pp, ff = np.meshgrid(np.arange(P), np.arange(P), indexing="ij")
        dups = [(pp == h * D + ff % D).astype(np.float32) for h in range(2)]
        _CONST["dup"] = np.concatenate(dups, axis=1)  # (128, 256)
    return _CONST


def _prep_core_inputs(dev, query, q_w, q_b, k_w, k_b, v_w, v_b, out_w):
    n = dev // 4
    hA = 2 * (dev % 4)
    a, b = hA * D, (hA + 1) * D
    cst = _const_tables()

    def pack_pe(w):
        # (128 feats, E) weight rows -> (p, e, f) stationary layout
        sel = np.concatenate([w[a:a + D, :], w[b:b + D, :]], axis=0)  # (128, E)
        return np.ascontiguousarray(
            sel.T.reshape(4, P, P).transpose(1, 0, 2))  # (p, e, f)

    x = query[:, n, :].astype(np.float32)  # (L, E)
    xT = np.ascontiguousarray(x.T.reshape(4, P, L).transpose(1, 0, 2))

    wqk = np.concatenate([pack_pe(q_w), pack_pe(k_w)], axis=2)  # (p, 4, 256)
    wa = np.concatenate([wqk.reshape(P, 1024), cst["dup"]], axis=1)
    wv = pack_pe(v_w)                                           # (p, 4, 128)
    outwT = np.concatenate(
        [out_w[:, a:a + D].T, out_w[:, b:b + D].T], axis=0)     # (128, 512)
    wbp = np.concatenate(
        [wv.reshape(P, 512), outwT, cst["ident"], cst["sc"]], axis=1)

    def bias_col(v):
        return np.concatenate([v[a:a + D], v[b:b + D]]).reshape(P, 1)

    cpk = np.concatenate(
        [cst["cosmask"], cst["s_col"], cst["c_col"],
         bias_col(q_b), bias_col(k_b), bias_col(v_b)],
        axis=1).astype(np.float32)                              # (128, 147)

    return {
        "wa": np.ascontiguousarray(wa).astype(BF16NP),
        "x0": np.ascontiguousarray(xT[:, :, :TCH]).astype(BF16NP),
        "wb": np.ascontiguousarray(wbp).astype(BF16NP),
        "x1": np.ascontiguousarray(xT[:, :, TCH:]).astype(BF16NP),
        "cp": np.ascontiguousarray(cpk),
    }


def run(inputs, trace=False, trace_kwargs=None):
    nc = _get_program()
    in_maps = [
        _prep_core_inputs(
            d, inputs["query"], inputs["q_w"], inputs["q_b"], inputs["k_w"],
            inputs["k_b"], inputs["v_w"], inputs["v_b"], inputs["out_w"])
        for d in range(NCORES)
    ]
    res = bass_utils.run_bass_kernel_spmd(
        nc, in_maps, list(range(NCORES)), trace=trace,
        **(trace_kwargs or {}),
    )
    parts = [res.results[i]["out"].astype(np.float32) for i in range(NCORES)]
    out0 = parts[0] + parts[1] + parts[2] + parts[3]
    out1 = parts[4] + parts[5] + parts[6] + parts[7]
    out = np.stack([out0, out1], axis=1) + inputs["out_b"][None, None, :]
    return out.astype(np.float32), res


def kernel(**inputs) -> np.ndarray:
    out, _ = run(inputs, trace=False)
    return out
